# revision 9
# baseline (speedup 1.0000x reference)
"""Self-contained Trainium2 Bass kernel for a 2-layer masked LSTM.

Problem (hardcoded): T=512, B=64, IN=512, H=1024, L=2, fp32 I/O.
Sharding: data-parallel over batch across 8 NeuronCores (B_local=8),
LSTM weights replicated. The time recurrence runs fully on-core.

Per-core device program (per layer):
  Phase 1 (xg):  xg[t,b,:] = x[t,b,:] @ Wih.T + (bih+bhh)  as a bulk
                 matmul (batch-major, x^T chunks stationary), written to
                 DRAM in bf16.
  Phase 2 (recurrence): per step t, gates are accumulated in PSUM
                 batch-major via column-tiled matmuls: an identity-pass
                 injects xg_t (+bias) into PSUM, then 8 k-chunk matmuls
                 add h_{t-1} @ Whh.T (h^T stationary [128,16], Whh^T
                 moving).  Four column groups (one per gate type i,f,o,g)
                 run concurrently on the PE array.  The PSUM gates are
                 converted to bf16 and DMA-transposed (xbar) to
                 gate-major [128, hc*16+q] layout, where the sigmoid/tanh
                 + cell update chain runs with 128-partition occupancy.
                 The state h is kept gate-major so it is directly the
                 next step's stationary operand.  Masking uses
                 copy_predicated (write-skip => NaN-proof junk lanes).
"""

import os
import sys
import numpy as np

os.environ.setdefault("NEURON_COMPILE_CACHE_URL", "/tmp/neuron_cc_cache")
os.environ.setdefault("NEURON_CC_FLAGS", "--cache_dir=/tmp/neuron_cc_cache")

sys.path.insert(0, "/opt/trn_rl_repo")

import ml_dtypes

T, B, IN, H, L = 512, 64, 512, 1024, 2
NCORES = 8
BL = B // NCORES          # 8 sequences per core
G4 = 4 * H                # 4096 gate rows
QP = 16                   # padded batch slots in gate-major layout (xbar rows)
HC = H // 128             # 8 h-chunks

BF16 = ml_dtypes.bfloat16

# dtype of the moving Whh operand / matmul N size
N_MOV = 512               # per-matmul moving free size (1 PSUM bank, fp32 out)


def _build_nc(t_steps=T, use_fori=True):
    import concourse.bass as bass
    import concourse.bacc as bacc
    import concourse.tile as tile
    import concourse.mybir as mybir
    from contextlib import ExitStack

    dt = mybir.dt
    AF = mybir.ActivationFunctionType
    OP = mybir.AluOpType

    nc = bacc.Bacc("TRN2", target_bir_lowering=False, debug=False)

    # ---------------- DRAM I/O ----------------
    def din(name, shape, dty):
        return nc.dram_tensor(name, list(shape), dty, kind="ExternalInput").ap()

    def dout(name, shape, dty):
        return nc.dram_tensor(name, list(shape), dty, kind="ExternalOutput").ap()

    def dtmp(name, shape, dty):
        return nc.dram_tensor(name, list(shape), dty).ap()

    xT = din("xT", (IN, t_steps * BL), dt.bfloat16)          # x transposed
    wihT = [din(f"wihT{l}", ((IN if l == 0 else H), G4), dt.bfloat16) for l in range(L)]
    whhT = [din(f"whhT{l}", (H, G4), dt.bfloat16) for l in range(L)]
    biasb = [din(f"biasb{l}", (128, G4), dt.bfloat16) for l in range(L)]
    mask_i = din("mask_i", (128, t_steps, HC * QP), dt.int8)
    ipad = din("ipad", (BL, 32), dt.bfloat16)                # eye(8) padded to 32 cols

    xg_d = [dtmp(f"xg{l}", (BL, t_steps, G4), dt.bfloat16) for l in range(L)]
    y0T = dtmp("y0T", (H, t_steps * BL), dt.bfloat16)        # layer0 output, transposed
    yT = dout("yT", (H, t_steps * BL), dt.float32)           # layer1 output, transposed
    hn = dout("hn", (L, 128, HC * QP), dt.float32)           # gate-major finals
    cn = dout("cn", (L, 128, HC * QP), dt.float32)

    TS = t_steps
    RC = (TS * BL) // 128                                     # row chunks in phase 1
    TPB = 32 if TS % 32 == 0 else TS                          # steps per loop body
    NBODY = TS // TPB

    with tile.TileContext(nc) as tc:
        with ExitStack() as ctx:
            # persistent pools
            wpool = ctx.enter_context(tc.tile_pool(name="weights", bufs=1))
            state = ctx.enter_context(tc.tile_pool(name="state", bufs=1))

            ipad_sb = wpool.tile([BL, 32], dt.bfloat16, tag="ipad")
            nc.sync.dma_start(ipad_sb[:], ipad[:])

            for l in range(L):
                d_in = IN if l == 0 else H
                kc_in = d_in // 128
                src_T = xT if l == 0 else y0T

                # ======== Phase 1: xg = x @ Wih.T + bias ========
                with ExitStack() as c1:
                    p1w = c1.enter_context(tc.tile_pool(name=f"p1w{l}", bufs=1))
                    p1x = c1.enter_context(tc.tile_pool(name=f"p1x{l}", bufs=3))
                    p1o = c1.enter_context(tc.tile_pool(name=f"p1o{l}", bufs=3))
                    p1p = c1.enter_context(
                        tc.tile_pool(name=f"p1p{l}", bufs=2, space="PSUM")
                    )

                    wih_sb = p1w.tile([128, kc_in * G4], dt.bfloat16, tag="wih")
                    for kc in range(kc_in):
                        nc.sync.dma_start(
                            wih_sb[:, kc * G4 : (kc + 1) * G4],
                            wihT[l][kc * 128 : (kc + 1) * 128, :],
                        )
                    bias_sb = p1w.tile([128, G4], dt.bfloat16, tag="bias")
                    nc.sync.dma_start(bias_sb[:], biasb[l][:])

                    for rc in range(RC):
                        xt = []
                        for kc in range(kc_in):
                            xtt = p1x.tile([128, 128], dt.bfloat16, tag=f"xt{kc}")
                            nc.sync.dma_start(
                                xtt[:],
                                src_T[
                                    kc * 128 : (kc + 1) * 128,
                                    rc * 128 : (rc + 1) * 128,
                                ],
                            )
                            xt.append(xtt)
                        for nh in range(G4 // 512):
                            ps = p1p.tile([128, 512], dt.float32, tag="ps")
                            for kc in range(kc_in):
                                nc.tensor.matmul(
                                    ps[:],
                                    xt[kc][:],
                                    wih_sb[:, kc * G4 + nh * 512 : kc * G4 + (nh + 1) * 512],
                                    start=(kc == 0),
                                    stop=(kc == kc_in - 1),
                                )
                            g = p1o.tile([128, 512], dt.bfloat16, tag="g")
                            nc.vector.scalar_tensor_tensor(
                                g[:], ps[:], 1.0,
                                bias_sb[:, nh * 512 : (nh + 1) * 512],
                                OP.mult, OP.add,
                            )
                            # rows of g are (t_local, b); dram wants (b, t, g)
                            nc.sync.dma_start(
                                xg_d[l][:, rc * 16 : (rc + 1) * 16,
                                        nh * 512 : (nh + 1) * 512]
                                .rearrange("b t g -> t b g"),
                                g[:],
                            )

                # ======== Phase 2: recurrence ========
                with ExitStack() as c2:
                    p2w = c2.enter_context(tc.tile_pool(name=f"p2w{l}", bufs=1))
                    p2xg = c2.enter_context(tc.tile_pool(name=f"p2xg{l}", bufs=4))
                    p2mk = c2.enter_context(tc.tile_pool(name=f"p2mk{l}", bufs=1))
                    p2ps = c2.enter_context(
                        tc.tile_pool(name=f"p2ps{l}", bufs=2, space="PSUM")
                    )
                    p2g = c2.enter_context(tc.tile_pool(name=f"p2g{l}", bufs=2))
                    p2t = c2.enter_context(tc.tile_pool(name=f"p2t{l}", bufs=2))
                    p2h = c2.enter_context(tc.tile_pool(name=f"p2h{l}", bufs=2))

                    whh_sb = p2w.tile([128, HC * G4], dt.bfloat16, tag="whh")
                    for kc in range(HC):
                        nc.sync.dma_start(
                            whh_sb[:, kc * G4 : (kc + 1) * G4],
                            whhT[l][kc * 128 : (kc + 1) * 128, :],
                        )

                    h_gm = state.tile([128, HC * QP], dt.bfloat16, tag=f"h{l}")
                    c_gm = state.tile([128, HC * QP], dt.float32, tag=f"c{l}")
                    nc.vector.memset(h_gm[:], 0.0)
                    nc.vector.memset(c_gm[:], 0.0)

                    hist_dt = dt.bfloat16 if l == 0 else dt.float32
                    ydst = y0T if l == 0 else yT

                    def emit_body(t0):
                        # t0: python int or ScalarValue (loop reg * TPB)
                        dyn = not isinstance(t0, int)
                        mki = p2mk.tile([128, TPB * 128], dt.int8, tag="mki")
                        if dyn:
                            msl = mask_i[:, bass.ds(t0, TPB), :]
                        else:
                            msl = mask_i[:, t0 : t0 + TPB, :]
                        nc.sync.dma_start(mki[:], msl)
                        hist = p2h.tile([128, TPB * 128], hist_dt, tag="hist")

                        for j_t in range(TPB):
                            if dyn:
                                xsl = xg_d[l][:, bass.ds(t0 + j_t, 1), :]
                            else:
                                xsl = xg_d[l][:, t0 + j_t : t0 + j_t + 1, :]
                            xg_t = p2xg.tile([BL, G4], dt.bfloat16, tag="xg")
                            nc.sync.dma_start(xg_t[:], xsl)

                            G = p2ps.tile([128, 1024], dt.float32, tag="G")
                            # identity pass: inject xg (+bias already folded)
                            for j in range(4):
                                for n2 in range(1024 // N_MOV):
                                    nsl = slice(n2 * N_MOV, (n2 + 1) * N_MOV)
                                    gc = j * 1024 + n2 * N_MOV
                                    nc.tensor.matmul(
                                        G[32 * j : 32 * j + 32, nsl],
                                        ipad_sb[:],
                                        xg_t[:, gc : gc + N_MOV],
                                        start=True, stop=False,
                                        tile_position=(0, 32 * j),
                                        skip_group_check=True,
                                    )
                            # h @ Whh.T, col-groups concurrent
                            for kc in range(HC):
                                for j in range(4):
                                    for n2 in range(1024 // N_MOV):
                                        nsl = slice(n2 * N_MOV, (n2 + 1) * N_MOV)
                                        gc = kc * G4 + j * 1024 + n2 * N_MOV
                                        nc.tensor.matmul(
                                            G[32 * j : 32 * j + QP, nsl],
                                            h_gm[:, kc * QP : (kc + 1) * QP],
                                            whh_sb[:, gc : gc + N_MOV],
                                            start=False, stop=(kc == HC - 1),
                                            tile_position=(0, 32 * j),
                                            skip_group_check=True,
                                        )

                            Gs = p2g.tile([128, 1024], dt.bfloat16, tag="Gs")
                            nc.vector.tensor_copy(Gs[:], G[:])

                            gm = p2g.tile([128, 512], dt.bfloat16, tag="gm")
                            for j in range(4):
                                nc.sync.dma_start(
                                    gm[:, j * 128 : (j + 1) * 128].rearrange(
                                        "p (hc q) -> p hc q", q=QP
                                    ),
                                    Gs[32 * j : 32 * j + QP, :],
                                    transpose=True,
                                )

                            sig = p2t.tile([128, 384], dt.float32, tag="sig")
                            nc.scalar.activation(sig[:], gm[:, 0:384], AF.Sigmoid)
                            tg = p2t.tile([128, 128], dt.float32, tag="tg")
                            nc.scalar.activation(tg[:], gm[:, 384:512], AF.Tanh)

                            a = p2t.tile([128, 128], dt.float32, tag="a")
                            nc.vector.tensor_mul(a[:], sig[:, 0:128], tg[:])
                            p2_ = p2t.tile([128, 128], dt.float32, tag="p2_")
                            nc.vector.tensor_mul(p2_[:], sig[:, 128:256], c_gm[:])
                            cc = p2t.tile([128, 128], dt.float32, tag="cc")
                            nc.vector.tensor_add(cc[:], a[:], p2_[:])

                            mq = j_t * 128
                            nc.vector.copy_predicated(
                                c_gm[:], mki[:, mq : mq + 128], cc[:]
                            )

                            th = p2t.tile([128, 128], dt.float32, tag="th")
                            nc.scalar.activation(th[:], c_gm[:], AF.Tanh)
                            h2 = p2t.tile([128, 128], dt.bfloat16, tag="h2")
                            nc.vector.tensor_mul(h2[:], sig[:, 256:384], th[:])
                            nc.vector.copy_predicated(
                                h_gm[:], mki[:, mq : mq + 128], h2[:]
                            )

                            nc.vector.tensor_copy(
                                hist[:, j_t * 128 : (j_t + 1) * 128], h_gm[:]
                            )

                        # flush history to DRAM (transposed layout)
                        yv = ydst.rearrange("(hc p) (t b) -> hc p t b", hc=HC, b=BL)
                        hv = hist[:].rearrange(
                            "p (t hc q) -> p t hc q", t=TPB, q=QP
                        )
                        for hc in range(HC):
                            if dyn:
                                ysl = yv[hc, :, bass.ds(t0, TPB), :]
                            else:
                                ysl = yv[hc, :, t0 : t0 + TPB, :]
                            nc.sync.dma_start(ysl, hv[:, :, hc, 0:BL])

                    if use_fori and NBODY > 1:
                        with tc.For_i(0, NBODY, 1) as ib:
                            emit_body(ib * TPB)
                    else:
                        for ib in range(NBODY):
                            emit_body(ib * TPB)

                    # finals
                    hf = p2t.tile([128, HC * QP], dt.float32, tag="hf")
                    nc.vector.tensor_copy(hf[:], h_gm[:])
                    nc.sync.dma_start(hn[l], hf[:])
                    nc.sync.dma_start(cn[l], c_gm[:])

    nc.compile()
    return nc


_NC_CACHE = {}


def _get_nc(t_steps=T, use_fori=True):
    key = (t_steps, use_fori)
    if key not in _NC_CACHE:
        _NC_CACHE[key] = _build_nc(t_steps, use_fori)
    return _NC_CACHE[key]


def _prep_core_inputs(core, input_, mask, Wp, t_steps=T):
    """Host-side layout prep for one core. Wp = per-layer permuted weights."""
    b0 = core * BL
    x_s = input_[:t_steps, b0 : b0 + BL, :]                  # [T, BL, IN]
    xT = np.ascontiguousarray(
        x_s.transpose(2, 0, 1).reshape(IN, t_steps * BL)
    ).astype(BF16)

    m_s = mask[:t_steps, b0 : b0 + BL]                       # [T, BL]
    mq = np.zeros((t_steps, HC, QP), np.float32)
    mq[:, :, :BL] = m_s[:, None, :]
    mq = mq.reshape(t_steps, HC * QP)
    mask_i = np.ascontiguousarray(
        np.broadcast_to(mq, (128, t_steps, HC * QP)).astype(np.int8)
    )

    ip = np.zeros((BL, 32), np.float32)
    ip[:, :BL] = np.eye(BL)
    ipad = ip.astype(BF16)

    d = {
        "xT": np.ascontiguousarray(xT),
        "mask_i": mask_i,
        "ipad": ipad,
    }
    for l in range(L):
        wihT, whhT, bias = Wp[l]
        d[f"wihT{l}"] = wihT
        d[f"whhT{l}"] = whhT
        d[f"biasb{l}"] = bias
    return d


def _permute_weights(Wih, Whh, bih, bhh):
    """Reorder gate rows (i,f,g,o) -> (i,f,o,g); transpose; cast bf16."""
    idx = np.concatenate(
        [np.arange(0, H), np.arange(H, 2 * H), np.arange(3 * H, 4 * H),
         np.arange(2 * H, 3 * H)]
    )
    wihT = np.ascontiguousarray(Wih[idx].T).astype(BF16)     # [d_in, 4H]
    whhT = np.ascontiguousarray(Whh[idx].T).astype(BF16)     # [H, 4H]
    bias = (bih + bhh)[idx].astype(np.float32)
    biasb = np.broadcast_to(bias, (128, G4)).astype(BF16)
    return wihT, whhT, np.ascontiguousarray(biasb)


def _decode_outputs(results, t_steps=T):
    y = np.empty((t_steps, B, H), np.float32)
    h_n = np.empty((L, B, H), np.float32)
    c_n = np.empty((L, B, H), np.float32)
    for core, r in enumerate(results):
        b0 = core * BL
        yT = r["yT"].reshape(H, t_steps, BL)
        y[:, b0 : b0 + BL, :] = yT.transpose(1, 2, 0)
        hn = r["hn"].reshape(L, 128, HC, QP)[:, :, :, :BL]    # [L,p,hc,b]
        cn = r["cn"].reshape(L, 128, HC, QP)[:, :, :, :BL]
        h_n[:, b0 : b0 + BL, :] = hn.transpose(0, 3, 2, 1).reshape(L, BL, H)
        c_n[:, b0 : b0 + BL, :] = cn.transpose(0, 3, 2, 1).reshape(L, BL, H)
    return y, (h_n, c_n)


def kernel(input_, mask, Wih0, Whh0, bih0, bhh0, Wih1, Whh1, bih1, bhh1,
           _t_steps=T, _use_fori=True, _trace=False):
    from concourse.bass_utils import run_bass_kernel_spmd

    input_ = np.asarray(input_, np.float32)
    mask = np.asarray(mask, np.float32)
    Wp = [
        _permute_weights(np.asarray(Wih0), np.asarray(Whh0),
                         np.asarray(bih0), np.asarray(bhh0)),
        _permute_weights(np.asarray(Wih1), np.asarray(Whh1),
                         np.asarray(bih1), np.asarray(bhh1)),
    ]
    nc = _get_nc(_t_steps, _use_fori)
    in_maps = [
        _prep_core_inputs(c, input_, mask, Wp, _t_steps) for c in range(NCORES)
    ]
    res = run_bass_kernel_spmd(
        nc, in_maps, list(range(NCORES)), trace=_trace
    )
    y, (h_n, c_n) = _decode_outputs(res.results, _t_steps)
    kernel._last_exec_ns = res.exec_time_ns
    return y, (h_n, c_n)


if __name__ == "__main__":
    # smoke build
    nc = _get_nc(32, True)
    print("built ok")
